# revision 24
# baseline (speedup 1.0000x reference)
"""Trainium2 Bass kernel for nn_LutLinear (BCQ/LUT-quantized linear layer).

Math (K=4096, N=4096, WBIT=3, GROUP=128, APOT=3):
  bits[k, b, n]  = bit (k%32) of binaryWeight[k//32, b, n]
  B              = 2*bits - 1                        (in {-1, +1})
  scale[n, b, g] = sum_a 2^alpha[n, b, g, a]
  out[n] = sum_{g,b} scale[n,b,g] * (sum_{k in group g} x[k] * B[k,b,n]) + bias[n]

Strategy (tensor-parallel over N, 8 cores, N'=512 each), raw bass (no Tile
framework -- manual semaphores, so the epilogue semaphore-clear churn that
dominated the Tile version's teardown disappears):
  * bw is DMA'd in two n-halves; DVE bit-unpack runs as 16 half-plane ops
    (zippered h0/h1 order) so unpacking starts while half 1 is in flight:
    (words << t) & 0x4040 on int16 lanes yields fp8e4 planes (0x40 = 2.0).
  * PE: 96 matmuls lhsT = block-diagonal x bank (bf16) [128, 32], rhs = fp8
    bit-plane view [128, 512] (stride 4).  The 3 b-matmuls per (s, c) target
    psum col-blocks 0/32/64 and column-tile 3-way on the array.
  * Tail: prod[q, n'] = psum96 * scale (bf16, one DVE op), ones^T @ prod on
    PE (97th row = bias2 = bias - sum_q scale*S_g), DVE copy psum->SBUF, DMA.
"""

import os
import sys

for _p in ("/opt/trn_rl_repo", "/opt/pypackages"):
    if os.path.isdir(_p) and _p not in sys.path:
        sys.path.insert(0, _p)

from contextlib import ExitStack

import ml_dtypes
import numpy as np

import concourse.bass as bass
from concourse import bacc, mybir
from concourse.bass_utils import run_bass_kernel_spmd

K = 4096
N = 4096
GROUP = 128
WBIT = 3
G = K // GROUP          # 32 groups
NCORES = 8
NS = N // NCORES        # 512 output features per core
WORDS = K // 32         # 128 packed words per (b, n)
Q = WBIT * G            # 96 psum rows
BF16 = ml_dtypes.bfloat16

_CACHE = {}


def _build(nc):
    f32 = mybir.dt.float32
    i32 = mybir.dt.int32
    i16 = mybir.dt.int16
    bf16 = mybir.dt.bfloat16
    f8 = mybir.dt.float8e4
    LSL = mybir.AluOpType.logical_shift_left
    LSR = mybir.AluOpType.logical_shift_right
    AND = mybir.AluOpType.bitwise_and

    bw = nc.dram_tensor("bw", [WORDS, WBIT * NS], i32, kind="ExternalInput")
    xall = nc.dram_tensor("xall", [WORDS, G * G], bf16, kind="ExternalInput")
    consts = nc.dram_tensor("consts", [Q, NS], bf16, kind="ExternalInput")
    bias2 = nc.dram_tensor("bias2", [1, NS], bf16, kind="ExternalInput")
    out = nc.dram_tensor("out", [1, NS], f32, kind="ExternalOutput")

    ctx = ExitStack()
    wsb = ctx.enter_context(nc.sbuf_tensor("wsb", [WORDS, WBIT * NS], i32))
    m16 = ctx.enter_context(nc.sbuf_tensor("m16", [128, 8 * 3072], i16))
    xsb = ctx.enter_context(nc.sbuf_tensor("xsb", [WORDS, G * G], bf16))
    csb = ctx.enter_context(nc.sbuf_tensor("csb", [Q, NS], bf16))
    pr = ctx.enter_context(nc.sbuf_tensor("pr", [Q + 1, NS], bf16))
    warm = ctx.enter_context(nc.sbuf_tensor("warm", [128, 1024], bf16))
    ones = ctx.enter_context(nc.sbuf_tensor("ones", [Q + 1, 1], bf16))
    outsb = ctx.enter_context(nc.sbuf_tensor("outsb", [1, NS], f32))
    ps96 = ctx.enter_context(nc.psum_tensor("ps96", [Q, NS], f32))
    psO = ctx.enter_context(nc.psum_tensor("psO", [1, NS], f32))

    s_bw0 = ctx.enter_context(nc.semaphore("s_bw0"))
    s_bw1 = ctx.enter_context(nc.semaphore("s_bw1"))
    s_b2 = ctx.enter_context(nc.semaphore("s_b2"))
    s_x = ctx.enter_context(nc.semaphore("s_x"))
    s_cs = ctx.enter_context(nc.semaphore("s_cs"))
    s_pool = ctx.enter_context(nc.semaphore("s_pool"))
    s_up = ctx.enter_context(nc.semaphore("s_up"))
    s_mm = ctx.enter_context(nc.semaphore("s_mm"))
    s_pr = ctx.enter_context(nc.semaphore("s_pr"))
    s_red = ctx.enter_context(nc.semaphore("s_red"))
    s_out = ctx.enter_context(nc.semaphore("s_out"))
    s_done = ctx.enter_context(nc.semaphore("s_done"))

    # Re-run safety: clear kernel semaphores before any engine proceeds.
    sem_nums = sorted(
        s.num
        for s in (s_bw0, s_bw1, s_b2, s_x, s_cs, s_pool, s_up, s_mm, s_pr, s_red, s_out, s_done)
    )
    for rng in _compact_ranges(sem_nums):
        nc.gpsimd.dma_reset(rng)
        nc.gpsimd.sem_clear(rng)
    nc._nrt_pseudo_barrier()

    w16 = wsb[:].bitcast(i16)                       # [128, 3072]; (h, b, n)
    xv = xsb[:].rearrange("p (j g) -> p j g", j=G)  # [128, 32, 32]

    # m16 slot s holds the fp8 plane as [h-half 3072 B][h-half 3072 B].
    def plane(h, s):
        c0 = 3072 * s + 1536 * h
        return m16[:, c0 : c0 + 1536]

    # rhs view for (s, b, c): [p, 2 (n-half), 256 (n)] fp8, flattened
    # free order (h, n) == natural psum columns 0..511.
    def rhs_view(s, b, c):
        slot = m16[:, 3072 * s : 3072 * (s + 1)].bitcast(f8)  # [128, 6144]
        v = slot.rearrange("p (h b n c) -> p b c h n", h=2, b=WBIT, n=NS // 2, c=4)
        return v[:, b, c, :, :]

    def unpack(eng, h, s):
        dst = plane(h, s)
        src_ = w16[:, 1536 * h : 1536 * (h + 1)]
        if s < 7:
            return eng.tensor_scalar(dst, src_, 6 - s, 0x4040, LSL, AND)
        return eng.tensor_scalar(dst, src_, 1, 0x4040, LSR, AND)

    with nc.Block(no_gpsimd_drain=True) as block:

        @block.sync
        def _(sync):
            half = WBIT * NS // 2  # 768 i32 cols per n-half
            sync.dma_start(wsb[:, 0:half], bw[:, 0:half]).then_inc(s_bw0, 16)
            sync.dma_start(wsb[:, half : 2 * half], bw[:, half : 2 * half]).then_inc(
                s_bw1, 16
            )
            sync.dma_start(pr[Q : Q + 1, :], bias2[0:1, :]).then_inc(s_b2, 16)
            sync.wait_ge(s_out, 1)
            sync.dma_start(out[0:1, :], outsb[:]).then_inc(s_done, 16)
            sync.wait_ge(s_done, 16)

        @block.scalar
        def _(scalar):
            scalar.dma_start(xsb[:], xall[:, :]).then_inc(s_x, 16)
            scalar.dma_start(csb[:], consts[:, :]).then_inc(s_cs, 16)

        @block.gpsimd
        def _(gpsimd):
            gpsimd.memset(warm[:], 0.0).then_inc(s_pool, 1)
            gpsimd.memset(ones[:], 1.0).then_inc(s_pool, 1)

        @block.vector
        def _(vector):
            order = [(0, 0), (0, 1)]
            for s in range(8):
                order += [(1, s)] + ([(0, s + 2)] if s + 2 < 8 else [])
            for h, s in order:
                if (h, s) == (0, 0):
                    vector.wait_ge(s_bw0, 16)
                elif (h, s) == (1, 0):
                    vector.wait_ge(s_bw1, 16)
                unpack(vector, h, s).then_inc(s_up, 1)
            vector.wait_ge(s_mm, WBIT)
            vector.wait_ge(s_cs, 16)
            vector.tensor_tensor(
                pr[0:Q, :], ps96[:], csb[:], mybir.AluOpType.mult
            ).then_inc(s_pr, 1)
            vector.wait_ge(s_red, 1)
            vector.tensor_scalar(
                outsb[:], psO[:], 0.0, None, mybir.AluOpType.add
            ).then_inc(s_out, 1)

        @block.tensor
        def _(tensor):
            tensor.wait_ge(s_pool, 1)
            wf32 = warm[:].bitcast(f32)             # [128, 512]
            tensor.matmul(
                psO[0:1, 0:512], wf32[:, 0:1], wf32[:, :], start=True, stop=True
            )
            tensor.matmul(
                psO[0:1, 0:512], wf32[:, 0:1], wf32[:, :], start=True, stop=True
            )
            tensor.matmul(
                psO[0:1, 0:512], warm[:, 0:1], warm[:, 0:512], start=True, stop=True
            )
            tensor.wait_ge(s_x, 16)
            for s in range(8):
                # h1 plane s is zipper op 2s+3 (s<7) / 16 (s=7)
                tensor.wait_ge(s_up, min(2 * s + 3, 16))
                for c in range(4):
                    j = 8 * c + s
                    for b in range(WBIT):
                        mm = tensor.matmul(
                            ps96[32 * b : 32 * b + 32, :],
                            xv[:, j, :],
                            rhs_view(s, b, c),
                            start=(s == 0 and c == 0),
                            stop=(s == 7 and c == 3),
                            skip_group_check=True,
                        )
                        if s == 7 and c == 3:
                            mm.then_inc(s_mm, 1)
            tensor.wait_ge(s_pr, 1)
            tensor.wait_ge(s_b2, 16)
            tensor.wait_ge(s_pool, 2)
            tensor.matmul(
                psO[0:1, :], ones[:, :], pr[:, :], start=True, stop=True
            ).then_inc(s_red, 1)

    ctx.close()


def _compact_ranges(nums):
    out = []
    start = prev = nums[0]
    for n in nums[1:]:
        if n == prev + 1:
            prev = n
            continue
        out.append(range(start, prev + 1))
        start = prev = n
    out.append(range(start, prev + 1))
    return out


def _get_nc():
    if "nc" not in _CACHE:
        nc = bacc.Bacc(
            "TRN2",
            target_bir_lowering=False,
            debug=False,
            enable_asserts=False,
            num_devices=1,
        )
        _build(nc)
        nc.compile()
        _CACHE["nc"] = nc
    return _CACHE["nc"]


def _prep_inputs(x, binaryWeight, alpha, bias):
    """Host-side shard + layout/encoding prep."""
    x = np.asarray(x, dtype=np.float32).reshape(K)
    binaryWeight = np.asarray(binaryWeight, dtype=np.int32)
    alpha = np.asarray(alpha, dtype=np.int32)
    bias = np.asarray(bias, dtype=np.float32).reshape(N)

    # Block-diagonal lhsT bank: xall[w, j*32 + g] = x[32w + j] iff g == w//4
    xall = np.zeros((WORDS, G, G), dtype=np.float32)  # [w, j, g]
    w = np.arange(WORDS)
    for j in range(G):
        xall[w, j, w // 4] = x[32 * w + j]
    xallb = xall.reshape(WORDS, G * G).astype(BF16)

    xb = xallb.astype(np.float32)
    sg = xb.reshape(WORDS, G, G).sum(axis=(0, 1))  # effective group sums [G]

    # scale[n, b, g] = sum_a 2^alpha (exact in bf16)
    scale = np.exp2(alpha.astype(np.float32)).sum(axis=-1)  # [N, WBIT, G]

    in_maps = []
    for cc in range(NCORES):
        nsl = slice(cc * NS, (cc + 1) * NS)
        bw_sh = np.ascontiguousarray(
            binaryWeight[:, :, nsl]
            .reshape(WORDS, WBIT, 2, NS // 2)
            .transpose(0, 2, 1, 3)
        ).reshape(WORDS, WBIT * NS)
        sc = scale[nsl]  # [NS, WBIT, G]
        consts = np.zeros((Q, NS), dtype=np.float32)
        for b in range(WBIT):
            consts[32 * b : 32 * b + 32, :] = sc[:, b, :].T
        b2 = bias[nsl] - np.einsum("nbg,g->n", sc, sg)
        in_maps.append(
            {
                "bw": bw_sh,
                "xall": xallb,
                "consts": consts.astype(BF16),
                "bias2": b2.reshape(1, NS).astype(BF16),
            }
        )
    return in_maps


def _run(inputs, trace=False, **kw):
    nc = _get_nc()
    in_maps = _prep_inputs(**inputs)
    res = run_bass_kernel_spmd(
        nc, in_maps, core_ids=list(range(NCORES)), trace=trace, **kw
    )
    outs = [res.results[cc]["out"].reshape(NS) for cc in range(NCORES)]
    full = np.concatenate(outs).reshape(1, N).astype(np.float32)
    return full, res


def kernel(**inputs):
    out, _ = _run(inputs, trace=False)
    return out

